# revision 6
# baseline (speedup 1.0000x reference)
"""Trainium2 Bass kernel for BinarySplitDecoder (binary-tree leaf probabilities).

Contract: kernel(x) takes the FULL input x [65536, 1023] fp32 and returns the
FULL output [65536, 1024] fp32 (leaf probabilities of a depth-10 binary split
tree, level-major node ordering).

Sharding: pure data parallel - batch dim split evenly across 8 NeuronCores.

Per-core kernel (rows_per_core = 8192, memory-bound):
  - fp16 I/O: the host converts x to fp16 and upcasts y back, halving HBM
    traffic to ~33.5 MB/core. Tolerance is 2e-2 relative to absmax; measured
    end-to-end error of the all-fp16 pipeline is ~1.5e-3.
  - Block (bit-reversal) layout: each level writes left-children into the
    first half and right-children into the second half of the next level's
    tile, so every DVE operand has a packed (stride-1) last dim. That keeps
    the ops out of the ~1.7x strided-write penalty AND qualifies them for the
    DVE 2x perf mode (2-byte dtype + packed). The resulting column order of
    y is bit-reversed; the host feeds alphas pre-permuted per level (so the
    device always reads them contiguously) and un-permutes y columns at the
    end. Both fixups are cheap numpy gathers on the host.
  - right = cur - left replaces right = cur * (1 - a): no "one minus x" pass,
    no oma tile, one DVE subtract instead.
  - Rows processed in chunks of g*128; partition p / free-group i holds batch
    row off + p*g + i, so every chunk DMA is one contiguous 2D block.
  - Loads issue from the ACT sequencer (HWDGE), stores from SP: each
    sequencer drains in order, so a store's wait (on DVE finishing chunk c)
    must not block later chunks' loads - splitting the queues decouples the
    two wait chains.
"""

import numpy as np

import concourse.bacc as bacc
import concourse.bass as bass
import concourse.mybir as mybir
from concourse.tile import TileContext
from concourse.bass_utils import run_bass_kernel_spmd

TREE_DEPTH = 10
N_NODES = (1 << TREE_DEPTH) - 1  # 1023
N_LEAVES = 1 << TREE_DEPTH  # 1024
N_CORES = 8
P = 128  # SBUF partitions


def _bitrev(j: int, bits: int) -> int:
    r = 0
    for _ in range(bits):
        r = (r << 1) | (j & 1)
        j >>= 1
    return r


def _input_perm() -> np.ndarray:
    """perm[k] = source column of x for device column k (level-major order,
    bit-reversed node index within each level)."""
    perm = np.empty(N_NODES, dtype=np.int64)
    for d in range(TREE_DEPTH):
        base = (1 << d) - 1
        for j in range(1 << d):
            perm[base + j] = base + _bitrev(j, d)
    return perm


def _output_perm() -> np.ndarray:
    """y[:, t] = y_dev[:, outperm[t]] (bit reversal, self-inverse)."""
    return np.array([_bitrev(t, TREE_DEPTH) for t in range(N_LEAVES)], dtype=np.int64)


_IN_PERM = _input_perm()
_OUT_PERM = _output_perm()


def build_nc(rows_per_core: int, G: int = 16, lead: tuple = (4, 4, 8)) -> bass.Bass:
    """Build the per-core Bass program.

    Reads DRAM input "x" [rows_per_core, 1023] fp16 (columns pre-permuted
    per level) and writes "y" [rows_per_core, 1024] fp16 (columns in
    bit-reversed leaf order).
    """
    units = rows_per_core // P
    lead = tuple(g for g in lead if g < G)
    if sum(lead) <= units and (units - sum(lead)) % G == 0:
        chunks = list(lead) + [G] * ((units - sum(lead)) // G)
    else:
        assert units % G == 0
        chunks = [G] * (units // G)
    assert sum(chunks) == units
    f16 = mybir.dt.float16

    nc = bacc.Bacc("TRN2", target_bir_lowering=False, debug=False)
    x = nc.declare_dram_parameter("x", [rows_per_core, N_NODES], f16, isOutput=False)
    y = nc.declare_dram_parameter("y", [rows_per_core, N_LEAVES], f16, isOutput=True)

    def x_view(off, g):
        return x[off : off + g * P, :].rearrange("(p g) n -> p (g n)", g=g, p=P)

    def y_view(off, g):
        return y[off : off + g * P, :].rearrange("(p g) m -> p (g m)", g=g, p=P)

    with TileContext(nc) as tc:
        with (
            tc.tile_pool(name="xin", bufs=2) as xp,
            tc.tile_pool(name="out", bufs=2) as outp,
            # bufs=2: with one buffer, chunk c+1's level-0 write must wait
            # for the level-9 reads of chunk c (WAR) - a per-chunk stall.
            tc.tile_pool(name="cur", bufs=2) as curp,
        ):
            # Two independent HWDGE queues per direction: descriptor issue
            # (~62ns each) caps a single queue near 264 GB/s, below the
            # 16-engine DMA wall. Alternating chunks across sequencers
            # doubles issue capacity. All four host engines are otherwise
            # (nearly) idle; compute stays on DVE.
            load_q = [nc.scalar, nc.gpsimd]
            store_q = [nc.sync, nc.sync]
            off = 0
            for c, g in enumerate(chunks):
                xt = xp.tile([P, g, N_NODES], f16, tag="x")
                load_q[c % 2].dma_start(out=xt[:], in_=x_view(off, g))

                out_t = outp.tile([P, g, N_LEAVES], f16, tag="y")
                cur = None
                for d in range(TREE_DEPTH):
                    L = 1 << d
                    if d == TREE_DEPTH - 1:
                        nxt = out_t
                    else:
                        # ping-pong intermediate levels between two shared
                        # slots (sized by the largest level using each tag)
                        nxt = curp.tile([P, g, 2 * L], f16, tag=f"cur{d % 2}")
                    a = xt[:, :, L - 1 : 2 * L - 1]  # [P, g, L] level-d alphas
                    left = nxt[:, :, 0:L]
                    right = nxt[:, :, L : 2 * L]
                    if d == 0:
                        # cur == 1:  left = a, right = 1 - a.
                        nc.vector.tensor_copy(out=left, in_=a)
                        nc.vector.tensor_scalar(
                            out=right,
                            in0=a,
                            scalar1=-1.0,
                            scalar2=1.0,
                            op0=mybir.AluOpType.mult,
                            op1=mybir.AluOpType.add,
                        )
                    else:
                        nc.vector.tensor_mul(out=left, in0=cur, in1=a)
                        nc.vector.tensor_sub(out=right, in0=cur, in1=left)
                    cur = nxt

                store_q[c % 2].dma_start(out=y_view(off, g), in_=out_t[:])
                off += g * P

    nc.compile()
    return nc


def _run(x: np.ndarray, **spmd_kwargs):
    """Shard x, run the Bass kernel on all 8 cores, return (y, BassKernelResults)."""
    x = np.asarray(x)
    B = x.shape[0]
    assert B % N_CORES == 0 and x.shape[1] == N_NODES
    rows_per_core = B // N_CORES

    # fp16 + per-level bit-reversed column order (see module docstring).
    x16 = np.ascontiguousarray(x.astype(np.float16)[:, _IN_PERM])

    nc = build_nc(rows_per_core)
    core_ids = list(range(N_CORES))
    in_maps = [
        {"x": x16[i * rows_per_core : (i + 1) * rows_per_core]} for i in core_ids
    ]
    res = run_bass_kernel_spmd(nc, in_maps, core_ids, **spmd_kwargs)
    y16 = np.concatenate([r["y"] for r in res.results], axis=0)
    out = y16[:, _OUT_PERM].astype(np.float32)
    return out, res


def kernel(x: np.ndarray) -> np.ndarray:
    return _run(x)[0]


# revision 12
# speedup vs baseline: 1.0522x; 1.0522x over previous
"""Trainium2 Bass kernel for BinarySplitDecoder (binary-tree leaf probabilities).

Contract: kernel(x) takes the FULL input x [65536, 1023] fp32 and returns the
FULL output [65536, 1024] fp32 (leaf probabilities of a depth-10 binary split
tree, level-major node ordering).

Sharding: pure data parallel - batch dim split evenly across 8 NeuronCores.

Per-core kernel (rows_per_core = 8192, memory-bound):
  - fp16 I/O: the host converts x to fp16 and upcasts y back, halving HBM
    traffic to ~33.5 MB/core. Tolerance is 2e-2 relative to absmax; measured
    end-to-end error of the all-fp16 pipeline is ~1.5e-3.
  - Block (bit-reversal) layout: each level writes left-children into the
    first half and right-children into the second half of the next level's
    tile, so every DVE operand has a packed (stride-1) last dim. That keeps
    the ops out of the ~1.7x strided-write penalty AND qualifies them for the
    DVE 2x perf mode (2-byte dtype + packed). The resulting column order of
    y is bit-reversed; the host feeds alphas pre-permuted per level (so the
    device always reads them contiguously) and un-permutes y columns at the
    end. Both fixups are cheap numpy gathers on the host.
  - right = cur - left replaces right = cur * (1 - a): no "one minus x" pass,
    no oma tile, one DVE subtract instead.
  - Rows processed in chunks of g*128; partition p / free-group i holds batch
    row off + p*g + i, so every chunk DMA is one contiguous 2D block.
  - Loads issue from the ACT sequencer (HWDGE), stores from SP: each
    sequencer drains in order, so a store's wait (on DVE finishing chunk c)
    must not block later chunks' loads - splitting the queues decouples the
    two wait chains.
"""

import numpy as np

import concourse.bacc as bacc
import concourse.bass as bass
import concourse.mybir as mybir
from concourse.tile import TileContext
from concourse.bass_utils import run_bass_kernel_spmd

TREE_DEPTH = 10
N_NODES = (1 << TREE_DEPTH) - 1  # 1023
N_LEAVES = 1 << TREE_DEPTH  # 1024
N_CORES = 8
P = 128  # SBUF partitions


def _bitrev(j: int, bits: int) -> int:
    r = 0
    for _ in range(bits):
        r = (r << 1) | (j & 1)
        j >>= 1
    return r


def _input_perm() -> np.ndarray:
    """perm[k] = source column of x for device column k (level-major order,
    bit-reversed node index within each level)."""
    perm = np.empty(N_NODES, dtype=np.int64)
    for d in range(TREE_DEPTH):
        base = (1 << d) - 1
        for j in range(1 << d):
            perm[base + j] = base + _bitrev(j, d)
    return perm


def _output_perm() -> np.ndarray:
    """y[:, t] = y_dev[:, outperm[t]] (bit reversal, self-inverse)."""
    return np.array([_bitrev(t, TREE_DEPTH) for t in range(N_LEAVES)], dtype=np.int64)


_IN_PERM = _input_perm()
_OUT_PERM = _output_perm()


def build_nc(rows_per_core: int, G: int = 8, lead: tuple = (4, 4)) -> bass.Bass:
    """Build the per-core Bass program.

    Reads DRAM input "x" [rows_per_core, 1023] fp16 (columns pre-permuted
    per level) and writes "y" [rows_per_core, 1024] fp16 (columns in
    bit-reversed leaf order).
    """
    units = rows_per_core // P
    lead = tuple(g for g in lead if g < G)
    tail = lead[::-1]  # small trailing chunks: the final store drains sooner
    body = units - sum(lead) - sum(tail)
    if body > 0 and body % G == 0:
        chunks = list(lead) + [G] * (body // G) + list(tail)
    else:
        assert units % G == 0
        chunks = [G] * (units // G)
    assert sum(chunks) == units
    f16 = mybir.dt.float16

    nc = bacc.Bacc("TRN2", target_bir_lowering=False, debug=False)
    x = nc.declare_dram_parameter("x", [rows_per_core, N_NODES], f16, isOutput=False)
    y = nc.declare_dram_parameter("y", [rows_per_core, N_LEAVES], f16, isOutput=True)

    def x_view(off, g):
        return x[off : off + g * P, :].rearrange("(p g) n -> p (g n)", g=g, p=P)

    def y_view(off, g):
        return y[off : off + g * P, :].rearrange("(p g) m -> p (g m)", g=g, p=P)

    with TileContext(nc) as tc:
        with (
            tc.tile_pool(name="xin", bufs=3) as xp,
            tc.tile_pool(name="out", bufs=3) as outp,
            # bufs=2: with one buffer, chunk c+1's level-0 write must wait
            # for the level-9 reads of chunk c (WAR) - a per-chunk stall.
            tc.tile_pool(name="cur", bufs=2) as curp,
        ):
            # Two independent HWDGE queues per direction: descriptor issue
            # (~62ns each) caps a single queue near 264 GB/s, below the
            # 16-engine DMA wall. Alternating chunks across sequencers
            # doubles issue capacity. All four host engines are otherwise
            # (nearly) idle; compute stays on DVE.
            load_q = [nc.scalar, nc.gpsimd]
            store_q = [nc.sync]
            off = 0
            for c, g in enumerate(chunks):
                xt = xp.tile([P, g, N_NODES], f16, tag="x")
                load_q[c % len(load_q)].dma_start(out=xt[:], in_=x_view(off, g))

                out_t = outp.tile([P, g, N_LEAVES], f16, tag="y")
                cur = None
                for d in range(TREE_DEPTH):
                    L = 1 << d
                    if d == TREE_DEPTH - 1:
                        nxt = out_t
                    else:
                        # ping-pong intermediate levels between two shared
                        # slots (sized by the largest level using each tag)
                        nxt = curp.tile([P, g, 2 * L], f16, tag=f"cur{d % 2}")
                    a = xt[:, :, L - 1 : 2 * L - 1]  # [P, g, L] level-d alphas
                    left = nxt[:, :, 0:L]
                    right = nxt[:, :, L : 2 * L]
                    if d == 0:
                        # cur == 1:  left = a, right = 1 - a.
                        nc.vector.tensor_copy(out=left, in_=a)
                        nc.vector.tensor_scalar(
                            out=right,
                            in0=a,
                            scalar1=-1.0,
                            scalar2=1.0,
                            op0=mybir.AluOpType.mult,
                            op1=mybir.AluOpType.add,
                        )
                    else:
                        nc.vector.tensor_mul(out=left, in0=cur, in1=a)
                        nc.vector.tensor_sub(out=right, in0=cur, in1=left)
                    cur = nxt

                store_q[c % len(store_q)].dma_start(out=y_view(off, g), in_=out_t[:])
                off += g * P

    nc.compile()
    return nc


def _run(x: np.ndarray, **spmd_kwargs):
    """Shard x, run the Bass kernel on all 8 cores, return (y, BassKernelResults)."""
    x = np.asarray(x)
    B = x.shape[0]
    assert B % N_CORES == 0 and x.shape[1] == N_NODES
    rows_per_core = B // N_CORES

    # fp16 + per-level bit-reversed column order (see module docstring).
    x16 = np.ascontiguousarray(x.astype(np.float16)[:, _IN_PERM])

    nc = build_nc(rows_per_core)
    core_ids = list(range(N_CORES))
    in_maps = [
        {"x": x16[i * rows_per_core : (i + 1) * rows_per_core]} for i in core_ids
    ]
    res = run_bass_kernel_spmd(nc, in_maps, core_ids, **spmd_kwargs)
    y16 = np.concatenate([r["y"] for r in res.results], axis=0)
    out = y16[:, _OUT_PERM].astype(np.float32)
    return out, res


def kernel(x: np.ndarray) -> np.ndarray:
    return _run(x)[0]


# revision 15
# speedup vs baseline: 1.1038x; 1.0490x over previous
"""Trainium2 Bass kernel for BinarySplitDecoder (binary-tree leaf probabilities).

Contract: kernel(x) takes the FULL input x [65536, 1023] fp32 and returns the
FULL output [65536, 1024] fp32 (leaf probabilities of a depth-10 binary split
tree, level-major node ordering).

Sharding: pure data parallel - batch dim split evenly across 8 NeuronCores.

Per-core kernel (rows_per_core = 8192, memory-bound):
  - fp16 I/O: the host converts x to fp16 and upcasts y back, halving HBM
    traffic to ~33.5 MB/core. Tolerance is 2e-2 relative to absmax; measured
    end-to-end error of the all-fp16 pipeline is ~1.5e-3.
  - Block (bit-reversal) layout: each level writes left-children into the
    first half and right-children into the second half of the next level's
    tile, so every DVE operand has a packed (stride-1) last dim. That keeps
    the ops out of the ~1.7x strided-write penalty AND qualifies them for the
    DVE 2x perf mode (2-byte dtype + packed). The resulting column order of
    y is bit-reversed; the host feeds alphas pre-permuted per level (so the
    device always reads them contiguously) and un-permutes y columns at the
    end. Both fixups are cheap numpy gathers on the host.
  - right = cur - left replaces right = cur * (1 - a): no "one minus x" pass,
    no oma tile, one DVE subtract instead.
  - Rows processed in chunks of g*128; partition p / free-group i holds batch
    row off + p*g + i, so every chunk DMA is one contiguous 2D block.
  - Loads issue from the ACT sequencer (HWDGE), stores from SP: each
    sequencer drains in order, so a store's wait (on DVE finishing chunk c)
    must not block later chunks' loads - splitting the queues decouples the
    two wait chains.
"""

import numpy as np

import concourse.bacc as bacc
import concourse.bass as bass
import concourse.mybir as mybir
from concourse.tile import TileContext
from concourse.bass_utils import run_bass_kernel_spmd

TREE_DEPTH = 10
N_NODES = (1 << TREE_DEPTH) - 1  # 1023
N_LEAVES = 1 << TREE_DEPTH  # 1024
N_CORES = 8
P = 128  # SBUF partitions


def _bitrev(j: int, bits: int) -> int:
    r = 0
    for _ in range(bits):
        r = (r << 1) | (j & 1)
        j >>= 1
    return r


def _input_perm() -> np.ndarray:
    """perm[k] = source column of x for device column k (level-major order,
    bit-reversed node index within each level)."""
    perm = np.empty(N_NODES, dtype=np.int64)
    for d in range(TREE_DEPTH):
        base = (1 << d) - 1
        for j in range(1 << d):
            perm[base + j] = base + _bitrev(j, d)
    return perm


def _output_perm() -> np.ndarray:
    """y[:, t] = y_dev[:, outperm[t]] (bit reversal, self-inverse)."""
    return np.array([_bitrev(t, TREE_DEPTH) for t in range(N_LEAVES)], dtype=np.int64)


_IN_PERM = _input_perm()
_OUT_PERM = _output_perm()


def build_nc(rows_per_core: int, G: int = 16, lead: tuple = (4, 4, 8)) -> bass.Bass:
    """Build the per-core Bass program.

    Reads DRAM input "x" [rows_per_core, 1023] fp16 (columns pre-permuted
    per level) and writes "y" [rows_per_core, 1024] fp16 (columns in
    bit-reversed leaf order).
    """
    units = rows_per_core // P
    lead = tuple(g for g in lead if g < G)
    tail = lead[::-1]  # small trailing chunks: the final store drains sooner
    body = units - sum(lead) - sum(tail)
    if body > 0 and body % G == 0:
        chunks = list(lead) + [G] * (body // G) + list(tail)
    else:
        assert units % G == 0
        chunks = [G] * (units // G)
    assert sum(chunks) == units
    f16 = mybir.dt.float16

    nc = bacc.Bacc("TRN2", target_bir_lowering=False, debug=False)
    x = nc.declare_dram_parameter("x", [rows_per_core, N_NODES], f16, isOutput=False)
    y = nc.declare_dram_parameter("y", [rows_per_core, N_LEAVES], f16, isOutput=True)

    def x_view(off, g):
        return x[off : off + g * P, :].rearrange("(p g) n -> p (g n)", g=g, p=P)

    def y_view(off, g):
        return y[off : off + g * P, :].rearrange("(p g) m -> p (g m)", g=g, p=P)

    with TileContext(nc) as tc:
        with (
            tc.tile_pool(name="xin", bufs=2) as xp,
            tc.tile_pool(name="out", bufs=2) as outp,
            # bufs=2: with one buffer, chunk c+1's level-0 write must wait
            # for the level-9 reads of chunk c (WAR) - a per-chunk stall.
            tc.tile_pool(name="cur", bufs=2) as curp,
        ):
            # Two independent HWDGE queues per direction: descriptor issue
            # (~62ns each) caps a single queue near 264 GB/s, below the
            # 16-engine DMA wall. Alternating chunks across sequencers
            # doubles issue capacity. All four host engines are otherwise
            # (nearly) idle; compute stays on DVE.
            load_q = [nc.scalar]
            store_q = [nc.sync, nc.gpsimd]
            off = 0
            for c, g in enumerate(chunks):
                xt = xp.tile([P, g, N_NODES], f16, tag="x")
                load_q[c % len(load_q)].dma_start(out=xt[:], in_=x_view(off, g))

                out_t = outp.tile([P, g, N_LEAVES], f16, tag="y")
                cur = None
                for d in range(TREE_DEPTH):
                    L = 1 << d
                    if d == TREE_DEPTH - 1:
                        nxt = out_t
                    else:
                        # ping-pong intermediate levels between two shared
                        # slots (sized by the largest level using each tag)
                        nxt = curp.tile([P, g, 2 * L], f16, tag=f"cur{d % 2}")
                    a = xt[:, :, L - 1 : 2 * L - 1]  # [P, g, L] level-d alphas
                    left = nxt[:, :, 0:L]
                    right = nxt[:, :, L : 2 * L]
                    if d == 0:
                        # cur == 1:  left = a, right = 1 - a.
                        nc.vector.tensor_copy(out=left, in_=a)
                        nc.vector.tensor_scalar(
                            out=right,
                            in0=a,
                            scalar1=-1.0,
                            scalar2=1.0,
                            op0=mybir.AluOpType.mult,
                            op1=mybir.AluOpType.add,
                        )
                    else:
                        nc.vector.tensor_mul(out=left, in0=cur, in1=a)
                        nc.vector.tensor_sub(out=right, in0=cur, in1=left)
                    cur = nxt

                store_q[c % len(store_q)].dma_start(out=y_view(off, g), in_=out_t[:])
                off += g * P

    nc.compile()
    return nc


def _run(x: np.ndarray, **spmd_kwargs):
    """Shard x, run the Bass kernel on all 8 cores, return (y, BassKernelResults)."""
    x = np.asarray(x)
    B = x.shape[0]
    assert B % N_CORES == 0 and x.shape[1] == N_NODES
    rows_per_core = B // N_CORES

    # fp16 + per-level bit-reversed column order (see module docstring).
    x16 = np.ascontiguousarray(x.astype(np.float16)[:, _IN_PERM])

    nc = build_nc(rows_per_core)
    core_ids = list(range(N_CORES))
    in_maps = [
        {"x": x16[i * rows_per_core : (i + 1) * rows_per_core]} for i in core_ids
    ]
    res = run_bass_kernel_spmd(nc, in_maps, core_ids, **spmd_kwargs)
    y16 = np.concatenate([r["y"] for r in res.results], axis=0)
    out = y16[:, _OUT_PERM].astype(np.float32)
    return out, res


def kernel(x: np.ndarray) -> np.ndarray:
    return _run(x)[0]


# revision 16
# speedup vs baseline: 1.1936x; 1.0813x over previous
"""Trainium2 Bass kernel for BinarySplitDecoder (binary-tree leaf probabilities).

Contract: kernel(x) takes the FULL input x [65536, 1023] fp32 and returns the
FULL output [65536, 1024] fp32 (leaf probabilities of a depth-10 binary split
tree, level-major node ordering).

Sharding: pure data parallel - batch dim split evenly across 8 NeuronCores.

Per-core kernel (rows_per_core = 8192, memory-bound at ~33.5 MB of fp16 HBM
I/O against a ~420 GB/s 16-engine DMA wall):
  - fp16 I/O: the host converts x to fp16 and upcasts y back, halving HBM
    traffic. Tolerance is 2e-2 relative to absmax; measured end-to-end error
    of the all-fp16 pipeline is ~1.5e-3.
  - Block (bit-reversal) layout: each level writes left-children into the
    first half and right-children into the second half of the next level's
    tile, so every DVE operand has a packed (stride-1) last dim. That avoids
    the ~1.7x strided-write penalty AND qualifies every tensor_tensor for
    the DVE 2x_1p perf mode (0.52 ns/elem/partition instead of 1.04). The
    resulting column order of y is bit-reversed; the host feeds alphas
    pre-permuted per level and un-permutes y columns at the end (cheap numpy
    gathers, not device work).
  - right = cur - left replaces right = cur * (1 - a): no "1 - x" pass.
  - Two passes: levels 0-5 run ONCE for all 8192 rows (partition p owns rows
    p*64..p*64+63) in 12 large DVE ops - the per-op sequencer overhead that
    dominated the small levels amortizes away. Levels 6-9 then run per row-
    chunk, pipelined against the loads of their alphas (xb) and the stores.
  - The level 0-5 alphas (xa, cols 0:63) and level 6-9 alphas (xb, cols
    63:1023) are separate DRAM params so both load fully contiguous.
  - Loads ride the ACT-sequencer HWDGE queue; stores alternate between the
    SP and GPSIMD queues (one store queue caps at ~210 GB/s of descriptor
    issue and becomes the tail; two drain in parallel and keep the 16 DMA
    engines fed together with the load queue).
  - Small leading/trailing chunks shorten the pipeline ramp and the final
    store tail.
"""

import numpy as np

import concourse.bacc as bacc
import concourse.bass as bass
import concourse.mybir as mybir
from concourse.tile import TileContext
from concourse.bass_utils import run_bass_kernel_spmd

TREE_DEPTH = 10
N_NODES = (1 << TREE_DEPTH) - 1  # 1023
N_LEAVES = 1 << TREE_DEPTH  # 1024
N_CORES = 8
P = 128  # SBUF partitions
SPLIT_D = 6  # levels < SPLIT_D run in pass A; levels >= SPLIT_D run in pass B
NA = (1 << SPLIT_D) - 1  # 63 alpha columns consumed by pass A
NB = N_NODES - NA  # 960 alpha columns consumed by pass B


def _bitrev(j: int, bits: int) -> int:
    r = 0
    for _ in range(bits):
        r = (r << 1) | (j & 1)
        j >>= 1
    return r


def _input_perm() -> np.ndarray:
    """perm[k] = source column of x for device column k (level-major order,
    bit-reversed node index within each level)."""
    perm = np.empty(N_NODES, dtype=np.int64)
    for d in range(TREE_DEPTH):
        base = (1 << d) - 1
        for j in range(1 << d):
            perm[base + j] = base + _bitrev(j, d)
    return perm


def _output_perm() -> np.ndarray:
    """y[:, t] = y_dev[:, outperm[t]] (bit reversal, self-inverse)."""
    return np.array([_bitrev(t, TREE_DEPTH) for t in range(N_LEAVES)], dtype=np.int64)


_IN_PERM = _input_perm()
_OUT_PERM = _output_perm()


def build_nc(rows_per_core: int, G: int = 8,
             lead: tuple = (4, 4), tail: tuple = (4, 2, 2)) -> bass.Bass:
    """Build the per-core Bass program.

    DRAM params (fp16, columns pre-permuted per level on the host):
      xa [rows, 63]   alphas for levels 0-5
      xb [rows, 960]  alphas for levels 6-9
      y  [rows, 1024] leaf probabilities, columns in bit-reversed order
    """
    U = rows_per_core // P  # row-units per partition; partition p owns
    # global rows p*U + u for u in [0, U)
    body = U - sum(lead) - sum(tail)
    assert body > 0 and body % G == 0
    chunks = list(lead) + [G] * (body // G) + list(tail)
    assert sum(chunks) == U
    f16 = mybir.dt.float16

    nc = bacc.Bacc("TRN2", target_bir_lowering=False, debug=False)
    xa = nc.declare_dram_parameter("xa", [rows_per_core, NA], f16, isOutput=False)
    xb = nc.declare_dram_parameter("xb", [rows_per_core, NB], f16, isOutput=False)
    y = nc.declare_dram_parameter("y", [rows_per_core, N_LEAVES], f16, isOutput=True)

    xb_v = xb.rearrange("(p u) n -> p u n", p=P, u=U)
    y_v = y.rearrange("(p u) m -> p u m", p=P, u=U)

    with TileContext(nc) as tc:
        with (
            tc.tile_pool(name="pre", bufs=1) as prep,
            tc.tile_pool(name="xin", bufs=3) as xp,
            tc.tile_pool(name="out", bufs=3) as outp,
            tc.tile_pool(name="cur", bufs=2) as curp,
        ):
            # ---- pass A: levels 0..5 for all rows, one shot ----
            xat = prep.tile([P, U, NA], f16, tag="xa")
            nc.scalar.dma_start(
                out=xat[:], in_=xa[:, :].rearrange("(p u) n -> p (u n)", p=P, u=U)
            )
            cur = None
            for d in range(SPLIT_D):
                L = 1 << d
                nxt = prep.tile([P, U, 2 * L], f16, tag=f"pre{d % 2}")
                a = xat[:, :, L - 1 : 2 * L - 1]
                left = nxt[:, :, 0:L]
                right = nxt[:, :, L : 2 * L]
                if d == 0:
                    nc.vector.tensor_copy(out=left, in_=a)
                    nc.vector.tensor_scalar(
                        out=right,
                        in0=a,
                        scalar1=-1.0,
                        scalar2=1.0,
                        op0=mybir.AluOpType.mult,
                        op1=mybir.AluOpType.add,
                    )
                else:
                    nc.vector.tensor_mul(out=left, in0=cur, in1=a)
                    nc.vector.tensor_sub(out=right, in0=cur, in1=left)
                cur = nxt
            curA = cur  # [P, U, 64] level-5 probabilities, persists for pass B

            # ---- pass B: levels 6..9, pipelined row chunks ----
            store_q = [nc.sync, nc.gpsimd]
            u0 = 0
            for c, g in enumerate(chunks):
                xt = xp.tile([P, g, NB], f16, tag="x")
                nc.scalar.dma_start(out=xt[:], in_=xb_v[:, u0 : u0 + g, :])

                out_t = outp.tile([P, g, N_LEAVES], f16, tag="y")
                cur = curA[:, u0 : u0 + g, :]
                col = 0
                for d in range(SPLIT_D, TREE_DEPTH):
                    L = 1 << d
                    if d == TREE_DEPTH - 1:
                        nxt = out_t
                    else:
                        nxt = curp.tile([P, g, 2 * L], f16, tag=f"cur{d % 2}")
                    a = xt[:, :, col : col + L]
                    col += L
                    left = nxt[:, :, 0:L]
                    right = nxt[:, :, L : 2 * L]
                    nc.vector.tensor_mul(out=left, in0=cur, in1=a)
                    nc.vector.tensor_sub(out=right, in0=cur, in1=left)
                    cur = nxt

                store_q[c % 2].dma_start(out=y_v[:, u0 : u0 + g, :], in_=out_t[:])
                u0 += g

    nc.compile()
    return nc


def _run(x: np.ndarray, **spmd_kwargs):
    """Shard x, run the Bass kernel on all 8 cores, return (y, BassKernelResults)."""
    x = np.asarray(x)
    B = x.shape[0]
    assert B % N_CORES == 0 and x.shape[1] == N_NODES
    rows_per_core = B // N_CORES

    # fp16 + per-level bit-reversed column order (see module docstring).
    x16 = x.astype(np.float16)[:, _IN_PERM]
    xa = np.ascontiguousarray(x16[:, :NA])
    xb = np.ascontiguousarray(x16[:, NA:])

    nc = build_nc(rows_per_core)
    core_ids = list(range(N_CORES))
    in_maps = [
        {
            "xa": xa[i * rows_per_core : (i + 1) * rows_per_core],
            "xb": xb[i * rows_per_core : (i + 1) * rows_per_core],
        }
        for i in core_ids
    ]
    res = run_bass_kernel_spmd(nc, in_maps, core_ids, **spmd_kwargs)
    y16 = np.concatenate([r["y"] for r in res.results], axis=0)
    out = y16[:, _OUT_PERM].astype(np.float32)
    return out, res


def kernel(x: np.ndarray) -> np.ndarray:
    return _run(x)[0]
